# revision 34
# baseline (speedup 1.0000x reference)
"""DTLN part-2 single streaming step on 8 TRN2 NeuronCores.

Structure of the reference:
    enc = enc_W @ y1                         # [256] matvec, 1MB weights
    instant-LN(enc) -> LSTM1 -> LSTM2        # 1.4MB weights, fully sequential
    mask = sigmoid(dense_W @ h2 + b)         # 128KB
    decoded = dec_W @ (mask * enc)           # [1024] matvec, 1MB weights

Sharding: the middle is sequential (global LN stats + LSTM state), so it is
replicated on every core.  Only the final dec_W matvec shards with zero
communication: core k owns output rows [128k, 128k+128) and the host
concatenates.

Layout: every vector is a column ([d<=128, 1] per tile, elements on
partitions).  Matvecs are PE matmuls with the weight tile stationary
(lhsT = W.T tile) and the activation column as the N=1 moving operand.
The instant-LN is dissolved algebraically so LSTM1 input matmuls start
straight from raw enc; mean/sum-sq reductions use a ones[128,128] matmul
(reduce + broadcast in one PE op).

Precision modes (WDT):
  "bf16"   — weights/activations in bf16, fp32 PSUM.  Fastest; output
             error ~3e-3 of scale.
  "bf16x2" — every matrix W is split hi = bf16(W), lo = bf16(W - hi) and
             every matvec input x into xhi/xlo likewise; each weight tile
             contributes Whi@xhi + Whi@xlo + Wlo@xhi (the dropped
             Wlo@xlo term is ~2^-18).  fp32-grade results (~1e-5) at
             bf16 matmul speed; twice the weight DMA bytes.

Outputs are converted to ROW layout on-chip before the store (dec via an
operand-swapped matmul, h/c via one identity matmul) — a [128,1] column
DMA fans out into 128 four-byte descriptors whose HBM write receipts
trickle in for ~7us, while a [1,128] row is one descriptor.

ACT tables: the Sqrt set is preloaded at t=0 (hidden under the DMA); the
Sigmoid/Tanh set loads right after the real LN sqrt, under the LSTM1
matmuls.  Small inputs go on the scalar-engine HWDGE ring as row-major
blobs (few fat descriptors) and are transposed on-chip with an embedded
identity.
"""

import numpy as np

FRAME = 1024
ENC = 256
HID = 128
EPS = 1e-7
NCORES = 8
P = 128

# "bf16" (fast, ~3e-3 of scale) | "bf16x2" (fp32-grade, ~20% slower)
WDT = "bf16x2"

# gate order i, f, o, g (PyTorch layout is i, f, g, o)
_PERM = np.concatenate([np.arange(0, 128), np.arange(128, 256),
                        np.arange(384, 512), np.arange(256, 384)])

_CACHE: dict = {}


def _build_bass():
    import concourse.bacc as bacc
    import concourse.mybir as mybir
    import concourse.tile as tile

    X2 = WDT == "bf16x2"
    f32 = mybir.dt.float32
    wdt = mybir.dt.bfloat16
    AFT = mybir.ActivationFunctionType
    ALU = mybir.AluOpType

    nc = bacc.Bacc()

    # weight blob column widths (hi block, then lo block when X2)
    W2 = 2 if X2 else 1
    NXR = 32 if X2 else 16      # xq blob rows
    d_vecs = nc.dram_tensor("vecs", [16, P + 16], f32, kind="ExternalInput")
    d_xq = nc.dram_tensor("xq", [NXR, P + NXR], wdt, kind="ExternalInput")
    d_we0 = nc.dram_tensor("we0", [P, 1024 * W2], wdt, kind="ExternalInput")
    d_we1 = nc.dram_tensor("we1", [P, 1024 * W2], wdt, kind="ExternalInput")
    d_wz1 = nc.dram_tensor("wz1", [P, 1536 * W2], wdt, kind="ExternalInput")
    d_wz2 = nc.dram_tensor("wz2", [P, 1024 * W2], wdt, kind="ExternalInput")
    d_wtl = nc.dram_tensor("wtl", [P, 512 * W2], wdt, kind="ExternalInput")
    d_ident = nc.dram_tensor("ident", [P, P], f32, kind="ExternalInput")
    d_hc = nc.dram_tensor("hc", [4, P], f32, kind="ExternalOutput")
    d_dec = nc.dram_tensor("dec", [1, P], f32, kind="ExternalOutput")

    with tile.TileContext(nc) as tc:
        with (
            tc.tile_pool(name="w", bufs=1) as wp,
            tc.tile_pool(name="s", bufs=1) as sp,
            tc.tile_pool(name="ps", bufs=1, space="PSUM") as pp,
        ):
            vecs_r = wp.tile([16, P + 16], f32, tag="vecs_r")
            xq_r = wp.tile([NXR, P + NXR], wdt, tag="xq_r")
            we0 = wp.tile([P, 1024 * W2], wdt, tag="we0")
            we1 = wp.tile([P, 1024 * W2], wdt, tag="we1")
            wz1 = wp.tile([P, 1536 * W2], wdt, tag="wz1")
            wz2 = wp.tile([P, 1024 * W2], wdt, tag="wz2")
            wtl = wp.tile([P, 512 * W2], wdt, tag="wtl")
            ident = wp.tile([P, P], f32, tag="ident")

            # Small inputs land ROW-major on the ACT HWDGE ring (few fat
            # descriptors; 128-descriptor column blobs would trickle
            # completions and steal SDMA attention from the weights).
            nc.scalar.dma_start(xq_r[:], d_xq[:])
            nc.scalar.dma_start(vecs_r[:], d_vecs[:])
            nc.sync.dma_start(we0[:], d_we0[:])
            nc.sync.dma_start(we1[:], d_we1[:])
            nc.sync.dma_start(wz1[:], d_wz1[:])
            nc.sync.dma_start(wz2[:], d_wz2[:])
            nc.sync.dma_start(wtl[:], d_wtl[:])
            nc.sync.dma_start(ident[:], d_ident[:])

            ones = sp.tile([P, P], wdt, tag="ones")
            nc.gpsimd.memset(ones[:], 1.0)
            konst = sp.tile([1, 1], f32, tag="konst")
            nc.gpsimd.memset(konst[:], 1.0)
            junk1 = sp.tile([1, 1], f32, tag="junk1")
            # Preload the Sqrt table set at t=0 (hidden under DMA); only
            # one ACT table set is resident at a time, so the Sigmoid set
            # is loaded right after the real LN sqrt (under the LSTM1 MMs).
            nc.scalar.activation(junk1[:], konst[:], AFT.Sqrt)

            # on-chip transpose of the small input blobs (PE idle here)
            xqT_ps = pp.tile([P, NXR], f32, tag="v2_ps")
            nc.tensor.matmul(xqT_ps[:], xq_r[0:NXR, 0:P],
                             xq_r[0:NXR, P:P + NXR], start=True, stop=True)
            xq = sp.tile([P, NXR], wdt, tag="xq")
            with nc.allow_low_precision("bf16 xq transpose"):
                nc.vector.tensor_copy(xq[:], xqT_ps[:])
            vecsT_ps = pp.tile([P, 16], f32, tag="u2_ps")
            nc.tensor.matmul(vecsT_ps[:], vecs_r[0:16, 0:P],
                             vecs_r[0:16, P:P + 16], start=True, stop=True)
            vecs = sp.tile([P, 16], f32, tag="vecs")
            nc.vector.tensor_copy(vecs[:], vecsT_ps[:])

            def mv(out, w, c0, width, xhi, xlo, start, stop):
                """one weight k-tile contribution (hi/lo aware).

                w[:, c0:c0+128] is the hi tile; in X2 mode the lo tile
                sits `width` columns later."""
                if not X2:
                    nc.tensor.matmul(out, w[:, c0:c0 + P], xhi,
                                     start=start, stop=stop)
                else:
                    nc.tensor.matmul(out, w[:, c0:c0 + P], xhi,
                                     start=start, stop=False)
                    nc.tensor.matmul(out, w[:, c0:c0 + P], xlo,
                                     start=False, stop=False)
                    nc.tensor.matmul(out, w[:, width + c0:width + c0 + P],
                                     xhi, start=False, stop=stop)

            def split(x_f32, tagbase, n=1):
                """device-side hi/lo split of a [P, n] f32 AP"""
                hi = sp.tile([P, n], wdt, tag=tagbase + "_hi")
                with nc.allow_low_precision("bf16 split"):
                    nc.vector.tensor_copy(hi[:], x_f32)
                if not X2:
                    return hi, None
                lo = sp.tile([P, n], wdt, tag=tagbase + "_lo")
                with nc.allow_low_precision("bf16 split"):
                    nc.vector.tensor_tensor(lo[:], x_f32, hi[:],
                                            ALU.subtract)
                return hi, lo

            # xq column indices (host packs hi block then lo block)
            XO = 16 if X2 else 0   # offset of the lo copies inside xq

            # ---- encoder: enc[256] = enc_W @ y1, columns of enc_ps ----
            enc_ps = pp.tile([P, 2], f32, tag="enc_ps")
            for m in range(2):
                for kk in range(8):
                    w = we0 if kk < 4 else we1
                    c = (kk % 4) * 256 + 128 * m
                    mv(enc_ps[:, m:m + 1], w, c, 1024,
                       xq[:, kk:kk + 1],
                       xq[:, XO + kk:XO + kk + 1] if X2 else None,
                       start=(kk == 0), stop=(kk == 7))
            enc_sb = sp.tile([P, 2], f32, tag="enc_sb")
            enc_hi = sp.tile([P, 2], wdt, tag="enc_hi")
            enc_lo = (sp.tile([P, 2], wdt, tag="enc_lo", name="enc_lo")
                      if X2 else None)
            for col in range(2):
                cs = slice(col, col + 1)
                with nc.allow_low_precision("bf16 split"):
                    nc.vector.tensor_copy(enc_hi[:, cs], enc_ps[:, cs])
                    if X2:
                        nc.vector.tensor_tensor(enc_lo[:, cs], enc_ps[:, cs],
                                                enc_hi[:, cs], ALU.subtract)
                nc.vector.tensor_copy(enc_sb[:, cs], enc_ps[:, cs])

            # ---- LN statistics: sum and sum-of-squares via ones-matmul ----
            st_ps = pp.tile([P, 2], f32, tag="st_ps")
            for kk in range(2):
                nc.tensor.matmul(st_ps[:, 0:1], ones[:],
                                 enc_hi[:, kk:kk + 1],
                                 start=(kk == 0),
                                 stop=(kk == 1) and not X2)
            if X2:
                for kk in range(2):
                    nc.tensor.matmul(st_ps[:, 0:1], ones[:],
                                     enc_lo[:, kk:kk + 1],
                                     start=False, stop=(kk == 1))
                sqf = sp.tile([P, 2], f32, tag="sqf")
                nc.vector.tensor_mul(sqf[:], enc_sb[:], enc_sb[:])
                sq_hi, sq_lo = split(sqf[:], "sq", 2)
            else:
                sq_hi = sp.tile([P, 2], wdt, tag="sq_hi")
                with nc.allow_low_precision("bf16 LN stats"):
                    nc.vector.tensor_mul(sq_hi[:], enc_hi[:], enc_hi[:])
                sq_lo = None
            for kk in range(2):
                nc.tensor.matmul(st_ps[:, 1:2], ones[:], sq_hi[:, kk:kk + 1],
                                 start=(kk == 0),
                                 stop=(kk == 1) and not X2)
            if X2:
                for kk in range(2):
                    nc.tensor.matmul(st_ps[:, 1:2], ones[:],
                                     sq_lo[:, kk:kk + 1],
                                     start=False, stop=(kk == 1))
            ms = sp.tile([P, 2], f32, tag="ms")  # [mean, E[x^2]]
            nc.scalar.activation(ms[:], st_ps[:], AFT.Identity,
                                 scale=1.0 / ENC)
            # m2e = mean^2 - EPS, so ve = E[x^2] - mean^2 + EPS
            m2 = sp.tile([P, 1], f32, tag="m2")
            nc.vector.tensor_scalar(m2[:], ms[:, 0:1], ms[:, 0:1], -EPS,
                                    ALU.mult, ALU.add)
            ve = sp.tile([P, 1], f32, tag="ve")
            nc.vector.tensor_sub(ve[:], ms[:, 1:2], m2[:])
            std = sp.tile([P, 1], f32, tag="std")
            nc.scalar.activation(std[:], ve[:], AFT.Sqrt)
            # kick the Sigmoid/Tanh table load now, under the LSTM1 MMs
            junk2 = sp.tile([1, 1], f32, tag="junk2")
            nc.scalar.activation(junk2[:], std[0:1, 0:1], AFT.Sigmoid)
            rstd = sp.tile([P, 1], f32, tag="rstd")
            nc.vector.reciprocal(rstd[:], std[:])
            ns = sp.tile([P, 1], f32, tag="ns")
            nc.vector.tensor_scalar(ns[:], ms[:, 0:1], rstd[:], -1.0,
                                    ALU.mult, ALU.mult)
            bias1 = sp.tile([P, 4], f32, tag="bias1")
            nc.vector.tensor_scalar(bias1[:], vecs[:, 8:12], ns[:], None,
                                    ALU.mult)
            bias1b = sp.tile([P, 4], f32, tag="bias1b")
            nc.vector.tensor_tensor(bias1b[:], bias1[:], vecs[:, 0:4],
                                    ALU.add)

            # ---- LSTM1: u1 = Wih1g@enc (raw enc!), v1 = Whh1@h1_in ----
            u1_ps = pp.tile([P, 4], f32, tag="u1_ps")
            for m in range(4):
                for kk in range(2):
                    c = 512 * kk + 128 * m
                    mv(u1_ps[:, m:m + 1], wz1, c, 1536,
                       enc_hi[:, kk:kk + 1],
                       enc_lo[:, kk:kk + 1] if X2 else None,
                       start=(kk == 0), stop=(kk == 1))
            v1_ps = pp.tile([P, 4], f32, tag="v1_ps")
            for m in range(4):
                c = 1024 + 128 * m
                mv(v1_ps[:, m:m + 1], wz1, c, 1536,
                   xq[:, 8:9], xq[:, XO + 8:XO + 9] if X2 else None,
                   start=True, stop=True)
            v1b = sp.tile([P, 4], f32, tag="v1b")
            nc.vector.tensor_tensor(v1b[:], v1_ps[:], bias1b[:], ALU.add)
            t0 = sp.tile([P, 4], f32, tag="t0")
            nc.vector.tensor_scalar(t0[:], u1_ps[:], rstd[:], None, ALU.mult)
            zin1 = sp.tile([P, 4], f32, tag="zin1")
            nc.vector.tensor_tensor(zin1[:], t0[:], v1b[:], ALU.add)
            g1 = sp.tile([P, 4], f32, tag="g1")
            nc.scalar.activation(g1[:, 0:3], zin1[:, 0:3], AFT.Sigmoid)
            nc.scalar.activation(g1[:, 3:4], zin1[:, 3:4], AFT.Tanh)

            hc = sp.tile([P, 4], f32, tag="hc")  # h1, c1, h2, c2 columns
            p1 = sp.tile([P, 1], f32, tag="p1")
            nc.vector.tensor_mul(p1[:], g1[:, 0:1], g1[:, 3:4])
            # c1_new = c1_in * f + i*g   (ACT-side FMA: Id(in*scale+bias))
            nc.scalar.activation(hc[:, 1:2], vecs[:, 4:5], AFT.Identity,
                                 bias=p1[:], scale=g1[:, 1:2])
            tc1 = sp.tile([P, 1], f32, tag="tc1")
            nc.scalar.activation(tc1[:], hc[:, 1:2], AFT.Tanh)
            h1f = sp.tile([P, 1], f32, tag="h1f")
            nc.vector.tensor_mul(h1f[:], g1[:, 2:3], tc1[:])
            h1_hi, h1_lo = split(h1f[:], "h1")
            nc.vector.tensor_copy(hc[:, 0:1], h1f[:])

            # ---- LSTM2: v2 = Whh2@h2_in runs early; u2 = Wih2@h1 ----
            v2_ps = pp.tile([P, 4], f32, tag="v2_ps")
            for m in range(4):
                c = 512 + 128 * m
                mv(v2_ps[:, m:m + 1], wz2, c, 1024,
                   xq[:, 9:10], xq[:, XO + 9:XO + 10] if X2 else None,
                   start=True, stop=True)
            e2 = sp.tile([P, 4], f32, tag="e2")
            nc.vector.tensor_tensor(e2[:], v2_ps[:], vecs[:, 12:16], ALU.add)
            u2_ps = pp.tile([P, 4], f32, tag="u2_ps")
            for m in range(4):
                mv(u2_ps[:, m:m + 1], wz2, 128 * m, 1024,
                   h1_hi[:], h1_lo[:] if X2 else None,
                   start=True, stop=True)
            zin2 = sp.tile([P, 4], f32, tag="zin2")
            nc.vector.tensor_tensor(zin2[:], u2_ps[:], e2[:], ALU.add)
            g2 = sp.tile([P, 4], f32, tag="g2")
            nc.scalar.activation(g2[:, 0:3], zin2[:, 0:3], AFT.Sigmoid)
            nc.scalar.activation(g2[:, 3:4], zin2[:, 3:4], AFT.Tanh)
            p2 = sp.tile([P, 1], f32, tag="p2")
            nc.vector.tensor_mul(p2[:], g2[:, 0:1], g2[:, 3:4])
            nc.scalar.activation(hc[:, 3:4], vecs[:, 5:6], AFT.Identity,
                                 bias=p2[:], scale=g2[:, 1:2])
            tc2 = sp.tile([P, 1], f32, tag="tc2")
            nc.scalar.activation(tc2[:], hc[:, 3:4], AFT.Tanh)
            h2f = sp.tile([P, 1], f32, tag="h2f")
            nc.vector.tensor_mul(h2f[:], g2[:, 2:3], tc2[:])
            h2_hi, h2_lo = split(h2f[:], "h2")
            nc.vector.tensor_copy(hc[:, 2:3], h2f[:])

            # ---- dense mask + decoder shard ----
            d_ps = pp.tile([P, 2], f32, tag="enc_ps")
            for m in range(2):
                mv(d_ps[:, m:m + 1], wtl, 128 * m, 512,
                   h2_hi[:], h2_lo[:] if X2 else None,
                   start=True, stop=True)
            msk = sp.tile([P, 2], f32, tag="msk")
            for m in range(2):
                nc.scalar.activation(msk[:, m:m + 1], d_ps[:, m:m + 1],
                                     AFT.Sigmoid, bias=vecs[:, 6 + m:7 + m])
            estf = sp.tile([P, 2], f32, tag="estf")
            nc.vector.tensor_mul(estf[:], msk[:], enc_sb[:])
            est_hi, est_lo = split(estf[:], "est", 2)
            # operand-swapped decoder matvec: out is a row [1, 128]
            o_ps = pp.tile([1, P], f32, tag="u1_ps")
            for kk in range(2):
                c = 256 + 128 * kk
                if not X2:
                    nc.tensor.matmul(o_ps[:], est_hi[:, kk:kk + 1],
                                     wtl[:, c:c + P],
                                     start=(kk == 0), stop=(kk == 1))
                else:
                    nc.tensor.matmul(o_ps[:], est_hi[:, kk:kk + 1],
                                     wtl[:, c:c + P],
                                     start=(kk == 0), stop=False)
                    nc.tensor.matmul(o_ps[:], est_lo[:, kk:kk + 1],
                                     wtl[:, c:c + P],
                                     start=False, stop=False)
                    nc.tensor.matmul(o_ps[:], est_hi[:, kk:kk + 1],
                                     wtl[:, 512 + c:512 + c + P],
                                     start=False, stop=(kk == 1))
            dec_sb = sp.tile([1, P], f32, tag="dec_sb")
            nc.vector.tensor_copy(dec_sb[:], o_ps[:])
            nc.scalar.dma_start(d_dec[:], dec_sb[:])

            # transpose h/c to rows via f32 identity matmul: [4,128] out
            # (emitted AFTER the dec chain so it doesn't delay the
            # critical dense->mask->dec path on the PE)
            hcT_ps = pp.tile([4, P], f32, tag="st_ps")
            nc.tensor.matmul(hcT_ps[:], hc[:, 0:4], ident[:],
                             start=True, stop=True)
            hcT = sp.tile([4, P], f32, tag="hcT")
            nc.vector.tensor_copy(hcT[:], hcT_ps[:])
            nc.sync.dma_start(d_hc[:], hcT[:])

    nc.compile()
    return nc


def _pack_inputs(inputs):
    """Host-side packing: transpose/permute weights into lhsT tile blobs."""
    import ml_dtypes
    bf = ml_dtypes.bfloat16
    X2 = WDT == "bf16x2"

    f = lambda x: np.ascontiguousarray(np.asarray(x, dtype=np.float32))
    y1 = f(inputs["y1"])
    h1_in, c1_in = f(inputs["h1_in"]), f(inputs["c1_in"])
    h2_in, c2_in = f(inputs["h2_in"]), f(inputs["c2_in"])
    enc_W = f(inputs["enc_W"])
    gamma, beta = f(inputs["gamma"]), f(inputs["beta"])
    Wih1, Whh1 = f(inputs["Wih1"]), f(inputs["Whh1"])
    bih1, bhh1 = f(inputs["bih1"]), f(inputs["bhh1"])
    Wih2, Whh2 = f(inputs["Wih2"]), f(inputs["Whh2"])
    bih2, bhh2 = f(inputs["bih2"]), f(inputs["bhh2"])
    dense_W, dense_b = f(inputs["dense_W"]), f(inputs["dense_b"])
    dec_W = f(inputs["dec_W"])

    def pack(w):
        """bf16 blob: hi block, then (X2) lo block, same layout."""
        hi = w.astype(bf)
        if not X2:
            return np.ascontiguousarray(hi)
        lo = (w - hi.astype(np.float32)).astype(bf)
        return np.ascontiguousarray(np.concatenate([hi, lo], axis=1))

    def hilo(v):
        hi = v.astype(bf)
        lo = (v - hi.astype(np.float32)).astype(bf)
        return hi.astype(np.float32), lo.astype(np.float32)

    G1 = Wih1 * gamma[None, :]
    Pg1 = G1[_PERM]                       # [512, 256] gate-permuted
    Ph1 = Whh1[_PERM]
    Pi2 = Wih2[_PERM]
    Ph2 = Whh2[_PERM]
    c1b = (Wih1 @ beta + bih1 + bhh1)[_PERM]
    c2b = (bih2 + bhh2)[_PERM]
    w1v = Pg1.sum(axis=1)                 # Wih1g @ ones

    vecs = np.zeros((16, P + 16), np.float32)
    vecs[0:4, 0:P] = c1b.reshape(4, P)
    vecs[4, 0:P] = c1_in
    vecs[5, 0:P] = c2_in
    vecs[6:8, 0:P] = dense_b.reshape(2, P)
    vecs[8:12, 0:P] = w1v.reshape(4, P)
    vecs[12:16, 0:P] = c2b.reshape(4, P)
    vecs[0:16, P:P + 16] = np.eye(16, dtype=np.float32)

    NXR = 32 if X2 else 16
    xq = np.zeros((NXR, P + NXR), np.float32)
    if X2:
        y_hi, y_lo = hilo(y1.reshape(8, P))
        h1h, h1l = hilo(h1_in)
        h2h, h2l = hilo(h2_in)
        xq[0:8, 0:P] = y_hi
        xq[8, 0:P] = h1h
        xq[9, 0:P] = h2h
        xq[16:24, 0:P] = y_lo
        xq[24, 0:P] = h1l
        xq[25, 0:P] = h2l
    else:
        xq[0:8, 0:P] = y1.reshape(8, P)
        xq[8, 0:P] = h1_in
        xq[9, 0:P] = h2_in
    xq[0:NXR, P:P + NXR] = np.eye(NXR, dtype=np.float32)

    eT = np.ascontiguousarray(enc_W.T).reshape(8, P, ENC)  # k-tiles
    we0 = np.concatenate([eT[i] for i in range(4)], axis=1)
    we1 = np.concatenate([eT[i] for i in range(4, 8)], axis=1)

    g1T = np.ascontiguousarray(Pg1.T).reshape(2, P, 512)
    wz1 = np.concatenate([g1T[0], g1T[1], Ph1.T], axis=1)  # [128, 1536]
    wz2 = np.concatenate([Pi2.T, Ph2.T], axis=1)           # [128, 1024]
    ident = np.eye(P, dtype=np.float32)

    in_maps = []
    for k in range(NCORES):
        Dk = dec_W[P * k:P * (k + 1), :]                   # [128, 256]
        dT = np.ascontiguousarray(Dk.T).reshape(2, P, P)
        wtl = np.concatenate([dense_W.T, dT[0], dT[1]], axis=1)  # [128, 512]
        in_maps.append({
            "vecs": vecs,
            "xq": np.ascontiguousarray(xq.astype(bf)),
            "we0": pack(we0),
            "we1": pack(we1),
            "wz1": pack(wz1),
            "wz2": pack(wz2),
            "wtl": pack(wtl),
            "ident": ident,
        })
    return in_maps


def _get_nc():
    if "nc" not in _CACHE:
        _CACHE["nc"] = _build_bass()
    return _CACHE["nc"]


def kernel(**inputs):
    from concourse.bass_utils import run_bass_kernel_spmd

    nc = _get_nc()
    in_maps = _pack_inputs(inputs)
    res = run_bass_kernel_spmd(nc, in_maps, list(range(NCORES))).results

    decoded = np.concatenate([res[k]["dec"][0, :] for k in range(NCORES)])
    hc = res[0]["hc"]
    return (
        decoded.reshape(1, FRAME, 1).astype(np.float32),
        hc[0].reshape(1, 1, HID).astype(np.float32),
        hc[1].reshape(1, 1, HID).astype(np.float32),
        hc[2].reshape(1, 1, HID).astype(np.float32),
        hc[3].reshape(1, 1, HID).astype(np.float32),
    )


# revision 35
# speedup vs baseline: 1.0728x; 1.0728x over previous
"""DTLN part-2 single streaming step on 8 TRN2 NeuronCores.

Structure of the reference:
    enc = enc_W @ y1                         # [256] matvec, 1MB weights
    instant-LN(enc) -> LSTM1 -> LSTM2        # 1.4MB weights, fully sequential
    mask = sigmoid(dense_W @ h2 + b)         # 128KB
    decoded = dec_W @ (mask * enc)           # [1024] matvec, 1MB weights

Sharding: the middle is sequential (global LN stats + LSTM state), so it is
replicated on every core.  Only the final dec_W matvec shards with zero
communication: core k owns output rows [128k, 128k+128) and the host
concatenates.

Layout: every vector is a column ([d<=128, 1] per tile, elements on
partitions).  Matvecs are PE matmuls with the weight tile stationary
(lhsT = W.T tile) and the activation column as the N=1 moving operand.
The instant-LN is dissolved algebraically so LSTM1 input matmuls start
straight from raw enc; mean/sum-sq reductions use a ones[128,128] matmul
(reduce + broadcast in one PE op).

Precision modes (WDT):
  "bf16"   — weights/activations in bf16, fp32 PSUM.  Fastest; output
             error ~3e-3 of scale.
  "bf16x2" — every matrix W is split hi = bf16(W), lo = bf16(W - hi) and
             every matvec input x into xhi/xlo likewise; each weight tile
             contributes Whi@xhi + Whi@xlo + Wlo@xhi (the dropped
             Wlo@xlo term is ~2^-18).  fp32-grade results (~1e-5) at
             bf16 matmul speed; twice the weight DMA bytes.

Outputs are converted to ROW layout on-chip before the store (dec via an
operand-swapped matmul, h/c via one identity matmul) — a [128,1] column
DMA fans out into 128 four-byte descriptors whose HBM write receipts
trickle in for ~7us, while a [1,128] row is one descriptor.

ACT tables: the Sqrt set is preloaded at t=0 (hidden under the DMA); the
Sigmoid/Tanh set loads right after the real LN sqrt, under the LSTM1
matmuls.  Small inputs go on the scalar-engine HWDGE ring as row-major
blobs (few fat descriptors) and are transposed on-chip with an embedded
identity.
"""

import numpy as np

FRAME = 1024
ENC = 256
HID = 128
EPS = 1e-7
NCORES = 8
P = 128

# "bf16" (fast, ~3e-3 of scale) | "bf16x2" (fp32-grade, ~20% slower)
WDT = "bf16x2"

# gate order i, f, o, g (PyTorch layout is i, f, g, o)
_PERM = np.concatenate([np.arange(0, 128), np.arange(128, 256),
                        np.arange(384, 512), np.arange(256, 384)])

_CACHE: dict = {}


def _build_bass():
    import concourse.bacc as bacc
    import concourse.mybir as mybir
    import concourse.tile as tile

    X2 = WDT == "bf16x2"
    f32 = mybir.dt.float32
    wdt = mybir.dt.bfloat16
    AFT = mybir.ActivationFunctionType
    ALU = mybir.AluOpType

    nc = bacc.Bacc()

    # weight blob column widths (hi block, then lo block when X2)
    W2 = 2 if X2 else 1
    NXR = 32 if X2 else 16      # xq blob rows
    d_vecs = nc.dram_tensor("vecs", [16, P + 16], f32, kind="ExternalInput")
    d_xq = nc.dram_tensor("xq", [NXR, P + NXR], wdt, kind="ExternalInput")
    d_we0 = nc.dram_tensor("we0", [P, 1024 * W2], wdt, kind="ExternalInput")
    d_we1 = nc.dram_tensor("we1", [P, 1024 * W2], wdt, kind="ExternalInput")
    d_wz1 = nc.dram_tensor("wz1", [P, 1536 * W2], wdt, kind="ExternalInput")
    d_wz2 = nc.dram_tensor("wz2", [P, 1024 * W2], wdt, kind="ExternalInput")
    d_wtl = nc.dram_tensor("wtl", [P, 512 * W2], wdt, kind="ExternalInput")
    d_ident = nc.dram_tensor("ident", [P, P], f32, kind="ExternalInput")
    d_hc = nc.dram_tensor("hc", [4, P], f32, kind="ExternalOutput")
    d_dec = nc.dram_tensor("dec", [1, P], f32, kind="ExternalOutput")

    with tile.TileContext(nc) as tc:
        with (
            tc.tile_pool(name="w", bufs=1) as wp,
            tc.tile_pool(name="s", bufs=1) as sp,
            tc.tile_pool(name="ps", bufs=1, space="PSUM") as pp,
        ):
            vecs_r = wp.tile([16, P + 16], f32, tag="vecs_r")
            xq_r = wp.tile([NXR, P + NXR], wdt, tag="xq_r")
            we0 = wp.tile([P, 1024 * W2], wdt, tag="we0")
            we1 = wp.tile([P, 1024 * W2], wdt, tag="we1")
            wz1 = wp.tile([P, 1536 * W2], wdt, tag="wz1")
            wz2 = wp.tile([P, 1024 * W2], wdt, tag="wz2")
            wtl = wp.tile([P, 512 * W2], wdt, tag="wtl")
            ident = wp.tile([P, P], f32, tag="ident")

            # Small inputs land ROW-major on the ACT HWDGE ring (few fat
            # descriptors; 128-descriptor column blobs would trickle
            # completions and steal SDMA attention from the weights).
            nc.scalar.dma_start(xq_r[:], d_xq[:])
            nc.scalar.dma_start(vecs_r[:], d_vecs[:])
            nc.sync.dma_start(we0[:], d_we0[:])
            nc.sync.dma_start(we1[:], d_we1[:])
            nc.sync.dma_start(wz1[:], d_wz1[:])
            nc.sync.dma_start(wz2[:], d_wz2[:])
            nc.sync.dma_start(wtl[:], d_wtl[:])
            nc.sync.dma_start(ident[:], d_ident[:])

            ones = sp.tile([P, P], wdt, tag="ones")
            nc.gpsimd.memset(ones[:], 1.0)
            konst = sp.tile([1, 1], f32, tag="konst")
            nc.gpsimd.memset(konst[:], 1.0)
            junk1 = sp.tile([1, 1], f32, tag="junk1")
            # Preload the Sqrt table set at t=0 (hidden under DMA); only
            # one ACT table set is resident at a time, so the Sigmoid set
            # is loaded right after the real LN sqrt (under the LSTM1 MMs).
            nc.scalar.activation(junk1[:], konst[:], AFT.Sqrt)

            # on-chip transpose of the small input blobs (PE idle here)
            xqT_ps = pp.tile([P, NXR], f32, tag="v2_ps")
            nc.tensor.matmul(xqT_ps[:], xq_r[0:NXR, 0:P],
                             xq_r[0:NXR, P:P + NXR], start=True, stop=True)
            xq = sp.tile([P, NXR], wdt, tag="xq")
            with nc.allow_low_precision("bf16 xq transpose"):
                nc.vector.tensor_copy(xq[:], xqT_ps[:])
            vecsT_ps = pp.tile([P, 16], f32, tag="u2_ps")
            nc.tensor.matmul(vecsT_ps[:], vecs_r[0:16, 0:P],
                             vecs_r[0:16, P:P + 16], start=True, stop=True)
            vecs = sp.tile([P, 16], f32, tag="vecs")
            nc.vector.tensor_copy(vecs[:], vecsT_ps[:])

            def mv(out, w, c0, width, xhi, xlo, start, stop):
                """one weight k-tile contribution (hi/lo aware).

                w[:, c0:c0+128] is the hi tile; in X2 mode the lo tile
                sits `width` columns later."""
                if not X2:
                    nc.tensor.matmul(out, w[:, c0:c0 + P], xhi,
                                     start=start, stop=stop)
                else:
                    nc.tensor.matmul(out, w[:, c0:c0 + P], xhi,
                                     start=start, stop=False)
                    nc.tensor.matmul(out, w[:, c0:c0 + P], xlo,
                                     start=False, stop=False)
                    nc.tensor.matmul(out, w[:, width + c0:width + c0 + P],
                                     xhi, start=False, stop=stop)

            def split(x_f32, tagbase, n=1):
                """device-side hi/lo split of a [P, n] f32 AP"""
                hi = sp.tile([P, n], wdt, tag=tagbase + "_hi")
                with nc.allow_low_precision("bf16 split"):
                    nc.vector.tensor_copy(hi[:], x_f32)
                if not X2:
                    return hi, None
                lo = sp.tile([P, n], wdt, tag=tagbase + "_lo")
                with nc.allow_low_precision("bf16 split"):
                    nc.vector.tensor_tensor(lo[:], x_f32, hi[:],
                                            ALU.subtract)
                return hi, lo

            # xq column indices (host packs hi block then lo block)
            XO = 16 if X2 else 0   # offset of the lo copies inside xq

            # ---- encoder: enc[256] = enc_W @ y1, columns of enc_ps ----
            enc_ps = pp.tile([P, 2], f32, tag="enc_ps")
            for m in range(2):
                for kk in range(8):
                    w = we0 if kk < 4 else we1
                    c = (kk % 4) * 256 + 128 * m
                    mv(enc_ps[:, m:m + 1], w, c, 1024,
                       xq[:, kk:kk + 1],
                       xq[:, XO + kk:XO + kk + 1] if X2 else None,
                       start=(kk == 0), stop=(kk == 7))
            enc_sb = sp.tile([P, 2], f32, tag="enc_sb")
            enc_hi, enc_lo = split(enc_ps[:], "enc", 2)
            nc.vector.tensor_copy(enc_sb[:], enc_ps[:])

            # ---- LN statistics: sum and sum-of-squares via ones-matmul ----
            st_ps = pp.tile([P, 2], f32, tag="st_ps")
            for kk in range(2):
                nc.tensor.matmul(st_ps[:, 0:1], ones[:],
                                 enc_hi[:, kk:kk + 1],
                                 start=(kk == 0),
                                 stop=(kk == 1) and not X2)
            if X2:
                for kk in range(2):
                    nc.tensor.matmul(st_ps[:, 0:1], ones[:],
                                     enc_lo[:, kk:kk + 1],
                                     start=False, stop=(kk == 1))
                sqf = sp.tile([P, 2], f32, tag="sqf")
                nc.vector.tensor_mul(sqf[:], enc_sb[:], enc_sb[:])
                sq_hi, sq_lo = split(sqf[:], "sq", 2)
            else:
                sq_hi = sp.tile([P, 2], wdt, tag="sq_hi")
                with nc.allow_low_precision("bf16 LN stats"):
                    nc.vector.tensor_mul(sq_hi[:], enc_hi[:], enc_hi[:])
                sq_lo = None
            for kk in range(2):
                nc.tensor.matmul(st_ps[:, 1:2], ones[:], sq_hi[:, kk:kk + 1],
                                 start=(kk == 0),
                                 stop=(kk == 1) and not X2)
            if X2:
                for kk in range(2):
                    nc.tensor.matmul(st_ps[:, 1:2], ones[:],
                                     sq_lo[:, kk:kk + 1],
                                     start=False, stop=(kk == 1))
            ms = sp.tile([P, 2], f32, tag="ms")  # [mean, E[x^2]]
            nc.scalar.activation(ms[:], st_ps[:], AFT.Identity,
                                 scale=1.0 / ENC)
            # m2e = mean^2 - EPS, so ve = E[x^2] - mean^2 + EPS
            m2 = sp.tile([P, 1], f32, tag="m2")
            nc.vector.tensor_scalar(m2[:], ms[:, 0:1], ms[:, 0:1], -EPS,
                                    ALU.mult, ALU.add)
            ve = sp.tile([P, 1], f32, tag="ve")
            nc.vector.tensor_sub(ve[:], ms[:, 1:2], m2[:])
            std = sp.tile([P, 1], f32, tag="std")
            nc.scalar.activation(std[:], ve[:], AFT.Sqrt)
            # kick the Sigmoid/Tanh table load now, under the LSTM1 MMs
            junk2 = sp.tile([1, 1], f32, tag="junk2")
            nc.scalar.activation(junk2[:], std[0:1, 0:1], AFT.Sigmoid)
            rstd = sp.tile([P, 1], f32, tag="rstd")
            nc.vector.reciprocal(rstd[:], std[:])
            ns = sp.tile([P, 1], f32, tag="ns")
            nc.vector.tensor_scalar(ns[:], ms[:, 0:1], rstd[:], -1.0,
                                    ALU.mult, ALU.mult)
            bias1 = sp.tile([P, 4], f32, tag="bias1")
            nc.vector.tensor_scalar(bias1[:], vecs[:, 8:12], ns[:], None,
                                    ALU.mult)
            bias1b = sp.tile([P, 4], f32, tag="bias1b")
            nc.vector.tensor_tensor(bias1b[:], bias1[:], vecs[:, 0:4],
                                    ALU.add)

            # ---- LSTM1: u1 = Wih1g@enc (raw enc!), v1 = Whh1@h1_in ----
            u1_ps = pp.tile([P, 4], f32, tag="u1_ps")
            for m in range(4):
                for kk in range(2):
                    c = 512 * kk + 128 * m
                    mv(u1_ps[:, m:m + 1], wz1, c, 1536,
                       enc_hi[:, kk:kk + 1],
                       enc_lo[:, kk:kk + 1] if X2 else None,
                       start=(kk == 0), stop=(kk == 1))
            v1_ps = pp.tile([P, 4], f32, tag="v1_ps")
            for m in range(4):
                c = 1024 + 128 * m
                mv(v1_ps[:, m:m + 1], wz1, c, 1536,
                   xq[:, 8:9], xq[:, XO + 8:XO + 9] if X2 else None,
                   start=True, stop=True)
            v1b = sp.tile([P, 4], f32, tag="v1b")
            nc.vector.tensor_tensor(v1b[:], v1_ps[:], bias1b[:], ALU.add)
            t0 = sp.tile([P, 4], f32, tag="t0")
            nc.vector.tensor_scalar(t0[:], u1_ps[:], rstd[:], None, ALU.mult)
            zin1 = sp.tile([P, 4], f32, tag="zin1")
            nc.vector.tensor_tensor(zin1[:], t0[:], v1b[:], ALU.add)
            g1 = sp.tile([P, 4], f32, tag="g1")
            nc.scalar.activation(g1[:, 0:3], zin1[:, 0:3], AFT.Sigmoid)
            nc.scalar.activation(g1[:, 3:4], zin1[:, 3:4], AFT.Tanh)

            hc = sp.tile([P, 4], f32, tag="hc")  # h1, c1, h2, c2 columns
            p1 = sp.tile([P, 1], f32, tag="p1")
            nc.vector.tensor_mul(p1[:], g1[:, 0:1], g1[:, 3:4])
            # c1_new = c1_in * f + i*g
            nc.vector.tensor_scalar(hc[:, 1:2], vecs[:, 4:5], g1[:, 1:2],
                                    p1[:], ALU.mult, ALU.add)
            tc1 = sp.tile([P, 1], f32, tag="tc1")
            nc.scalar.activation(tc1[:], hc[:, 1:2], AFT.Tanh)
            h1f = sp.tile([P, 1], f32, tag="h1f")
            nc.vector.tensor_mul(h1f[:], g1[:, 2:3], tc1[:])
            h1_hi, h1_lo = split(h1f[:], "h1")
            nc.vector.tensor_copy(hc[:, 0:1], h1f[:])

            # ---- LSTM2: v2 = Whh2@h2_in runs early; u2 = Wih2@h1 ----
            v2_ps = pp.tile([P, 4], f32, tag="v2_ps")
            for m in range(4):
                c = 512 + 128 * m
                mv(v2_ps[:, m:m + 1], wz2, c, 1024,
                   xq[:, 9:10], xq[:, XO + 9:XO + 10] if X2 else None,
                   start=True, stop=True)
            e2 = sp.tile([P, 4], f32, tag="e2")
            nc.vector.tensor_tensor(e2[:], v2_ps[:], vecs[:, 12:16], ALU.add)
            u2_ps = pp.tile([P, 4], f32, tag="u2_ps")
            for m in range(4):
                mv(u2_ps[:, m:m + 1], wz2, 128 * m, 1024,
                   h1_hi[:], h1_lo[:] if X2 else None,
                   start=True, stop=True)
            zin2 = sp.tile([P, 4], f32, tag="zin2")
            nc.vector.tensor_tensor(zin2[:], u2_ps[:], e2[:], ALU.add)
            g2 = sp.tile([P, 4], f32, tag="g2")
            nc.scalar.activation(g2[:, 0:3], zin2[:, 0:3], AFT.Sigmoid)
            nc.scalar.activation(g2[:, 3:4], zin2[:, 3:4], AFT.Tanh)
            p2 = sp.tile([P, 1], f32, tag="p2")
            nc.vector.tensor_mul(p2[:], g2[:, 0:1], g2[:, 3:4])
            nc.vector.tensor_scalar(hc[:, 3:4], vecs[:, 5:6], g2[:, 1:2],
                                    p2[:], ALU.mult, ALU.add)
            tc2 = sp.tile([P, 1], f32, tag="tc2")
            nc.scalar.activation(tc2[:], hc[:, 3:4], AFT.Tanh)
            h2f = sp.tile([P, 1], f32, tag="h2f")
            nc.vector.tensor_mul(h2f[:], g2[:, 2:3], tc2[:])
            h2_hi, h2_lo = split(h2f[:], "h2")
            nc.vector.tensor_copy(hc[:, 2:3], h2f[:])

            # ---- dense mask + decoder shard ----
            d_ps = pp.tile([P, 2], f32, tag="enc_ps")
            for m in range(2):
                mv(d_ps[:, m:m + 1], wtl, 128 * m, 512,
                   h2_hi[:], h2_lo[:] if X2 else None,
                   start=True, stop=True)
            msk = sp.tile([P, 2], f32, tag="msk")
            for m in range(2):
                nc.scalar.activation(msk[:, m:m + 1], d_ps[:, m:m + 1],
                                     AFT.Sigmoid, bias=vecs[:, 6 + m:7 + m])
            estf = sp.tile([P, 2], f32, tag="estf")
            nc.vector.tensor_mul(estf[:], msk[:], enc_sb[:])
            est_hi, est_lo = split(estf[:], "est", 2)
            # operand-swapped decoder matvec: out is a row [1, 128]
            o_ps = pp.tile([1, P], f32, tag="u1_ps")
            for kk in range(2):
                c = 256 + 128 * kk
                if not X2:
                    nc.tensor.matmul(o_ps[:], est_hi[:, kk:kk + 1],
                                     wtl[:, c:c + P],
                                     start=(kk == 0), stop=(kk == 1))
                else:
                    nc.tensor.matmul(o_ps[:], est_hi[:, kk:kk + 1],
                                     wtl[:, c:c + P],
                                     start=(kk == 0), stop=False)
                    nc.tensor.matmul(o_ps[:], est_lo[:, kk:kk + 1],
                                     wtl[:, c:c + P],
                                     start=False, stop=False)
                    nc.tensor.matmul(o_ps[:], est_hi[:, kk:kk + 1],
                                     wtl[:, 512 + c:512 + c + P],
                                     start=False, stop=(kk == 1))
            dec_sb = sp.tile([1, P], f32, tag="dec_sb")
            nc.vector.tensor_copy(dec_sb[:], o_ps[:])
            nc.scalar.dma_start(d_dec[:], dec_sb[:])

            # transpose h/c to rows via f32 identity matmul: [4,128] out
            # (emitted AFTER the dec chain so it doesn't delay the
            # critical dense->mask->dec path on the PE)
            hcT_ps = pp.tile([4, P], f32, tag="st_ps")
            nc.tensor.matmul(hcT_ps[:], hc[:, 0:4], ident[:],
                             start=True, stop=True)
            hcT = sp.tile([4, P], f32, tag="hcT")
            nc.vector.tensor_copy(hcT[:], hcT_ps[:])
            nc.sync.dma_start(d_hc[:], hcT[:])

    nc.compile()
    return nc


def _pack_inputs(inputs):
    """Host-side packing: transpose/permute weights into lhsT tile blobs."""
    import ml_dtypes
    bf = ml_dtypes.bfloat16
    X2 = WDT == "bf16x2"

    f = lambda x: np.ascontiguousarray(np.asarray(x, dtype=np.float32))
    y1 = f(inputs["y1"])
    h1_in, c1_in = f(inputs["h1_in"]), f(inputs["c1_in"])
    h2_in, c2_in = f(inputs["h2_in"]), f(inputs["c2_in"])
    enc_W = f(inputs["enc_W"])
    gamma, beta = f(inputs["gamma"]), f(inputs["beta"])
    Wih1, Whh1 = f(inputs["Wih1"]), f(inputs["Whh1"])
    bih1, bhh1 = f(inputs["bih1"]), f(inputs["bhh1"])
    Wih2, Whh2 = f(inputs["Wih2"]), f(inputs["Whh2"])
    bih2, bhh2 = f(inputs["bih2"]), f(inputs["bhh2"])
    dense_W, dense_b = f(inputs["dense_W"]), f(inputs["dense_b"])
    dec_W = f(inputs["dec_W"])

    def pack(w):
        """bf16 blob: hi block, then (X2) lo block, same layout."""
        hi = w.astype(bf)
        if not X2:
            return np.ascontiguousarray(hi)
        lo = (w - hi.astype(np.float32)).astype(bf)
        return np.ascontiguousarray(np.concatenate([hi, lo], axis=1))

    def hilo(v):
        hi = v.astype(bf)
        lo = (v - hi.astype(np.float32)).astype(bf)
        return hi.astype(np.float32), lo.astype(np.float32)

    G1 = Wih1 * gamma[None, :]
    Pg1 = G1[_PERM]                       # [512, 256] gate-permuted
    Ph1 = Whh1[_PERM]
    Pi2 = Wih2[_PERM]
    Ph2 = Whh2[_PERM]
    c1b = (Wih1 @ beta + bih1 + bhh1)[_PERM]
    c2b = (bih2 + bhh2)[_PERM]
    w1v = Pg1.sum(axis=1)                 # Wih1g @ ones

    vecs = np.zeros((16, P + 16), np.float32)
    vecs[0:4, 0:P] = c1b.reshape(4, P)
    vecs[4, 0:P] = c1_in
    vecs[5, 0:P] = c2_in
    vecs[6:8, 0:P] = dense_b.reshape(2, P)
    vecs[8:12, 0:P] = w1v.reshape(4, P)
    vecs[12:16, 0:P] = c2b.reshape(4, P)
    vecs[0:16, P:P + 16] = np.eye(16, dtype=np.float32)

    NXR = 32 if X2 else 16
    xq = np.zeros((NXR, P + NXR), np.float32)
    if X2:
        y_hi, y_lo = hilo(y1.reshape(8, P))
        h1h, h1l = hilo(h1_in)
        h2h, h2l = hilo(h2_in)
        xq[0:8, 0:P] = y_hi
        xq[8, 0:P] = h1h
        xq[9, 0:P] = h2h
        xq[16:24, 0:P] = y_lo
        xq[24, 0:P] = h1l
        xq[25, 0:P] = h2l
    else:
        xq[0:8, 0:P] = y1.reshape(8, P)
        xq[8, 0:P] = h1_in
        xq[9, 0:P] = h2_in
    xq[0:NXR, P:P + NXR] = np.eye(NXR, dtype=np.float32)

    eT = np.ascontiguousarray(enc_W.T).reshape(8, P, ENC)  # k-tiles
    we0 = np.concatenate([eT[i] for i in range(4)], axis=1)
    we1 = np.concatenate([eT[i] for i in range(4, 8)], axis=1)

    g1T = np.ascontiguousarray(Pg1.T).reshape(2, P, 512)
    wz1 = np.concatenate([g1T[0], g1T[1], Ph1.T], axis=1)  # [128, 1536]
    wz2 = np.concatenate([Pi2.T, Ph2.T], axis=1)           # [128, 1024]
    ident = np.eye(P, dtype=np.float32)

    in_maps = []
    for k in range(NCORES):
        Dk = dec_W[P * k:P * (k + 1), :]                   # [128, 256]
        dT = np.ascontiguousarray(Dk.T).reshape(2, P, P)
        wtl = np.concatenate([dense_W.T, dT[0], dT[1]], axis=1)  # [128, 512]
        in_maps.append({
            "vecs": vecs,
            "xq": np.ascontiguousarray(xq.astype(bf)),
            "we0": pack(we0),
            "we1": pack(we1),
            "wz1": pack(wz1),
            "wz2": pack(wz2),
            "wtl": pack(wtl),
            "ident": ident,
        })
    return in_maps


def _get_nc():
    if "nc" not in _CACHE:
        _CACHE["nc"] = _build_bass()
    return _CACHE["nc"]


def kernel(**inputs):
    from concourse.bass_utils import run_bass_kernel_spmd

    nc = _get_nc()
    in_maps = _pack_inputs(inputs)
    res = run_bass_kernel_spmd(nc, in_maps, list(range(NCORES))).results

    decoded = np.concatenate([res[k]["dec"][0, :] for k in range(NCORES)])
    hc = res[0]["hc"]
    return (
        decoded.reshape(1, FRAME, 1).astype(np.float32),
        hc[0].reshape(1, 1, HID).astype(np.float32),
        hc[1].reshape(1, 1, HID).astype(np.float32),
        hc[2].reshape(1, 1, HID).astype(np.float32),
        hc[3].reshape(1, 1, HID).astype(np.float32),
    )
